# revision 3
# baseline (speedup 1.0000x reference)
"""Trainium2 Bass kernel for CausalSelfAttention (B=4, T=2048, C=1024, H=16)
with additive prev-prob key bias.

Sharding: 8 cores = data-parallel over B (4) x tensor-parallel over head
halves (2).  Each core computes qkv for its 8 heads, causal attention, and a
partial output projection (row-parallel W_proj); host sums the two partials
per batch at unshard time.

Per-core device algorithm (v2 — scheduled for warm-HAM dense tensor queue):
  - All matmul operands are bf16 (host-cast inputs); PSUM accumulation stays
    fp32, and scores enter exp at fp32 PSUM precision.  bf16 enables FWL
    weight loads (hidden LDWEIGHTS) and halves input DMA + SBUF footprint.
  - K^T and Q^T kept feature-major with head pairs stacked in the 128
    partitions; QK^T runs as a row-tiled concurrent pair (K=64 at base
    partitions 0/64) — one 512-col pass for both heads.
  - Scores are computed transposed (keys on partitions): softmax denominator
    comes from an extra EA column appended to V (M=65 PV matmuls), where
    EA[k] = (prev_probs[k]+1e-10)**-EPS folds the additive log bias into a
    multiplicative per-key scale of exp(qk/8).
  - Causality: block-trimmed matmul widths + one 128x128 triangular mask
    multiply per diagonal block (DVE, bf16).
  - The scalar (ACT) engine runs a pure exp stream — y/qt/kt evacuations all
    live on DVE so exp is never queued behind work that waits on the
    reciprocal bounce.  PV is software-pipelined two score-tiles behind QK so
    exp latency never stalls the in-order tensor queue; QKV-gen of the next
    chunk and projection of the previous chunk fill the remaining slack via
    a deficit-driven pull.
  - Per-(head,query) 1/denominator via DRAM partition-scatter bounce on the
    gpsimd SWDGE queue (off-critical thanks to deep tmp/sc pools).
"""

import math
from contextlib import ExitStack

import numpy as np

import concourse.bass as bass
import concourse.tile as tile
from concourse import bacc, mybir

F32 = mybir.dt.float32
BF16 = mybir.dt.bfloat16

B, T, C, H = 4, 2048, 1024, 16
HD = C // H          # 64
NCORES = 8
HPC = H // 2         # 8 heads per core
FPC = HPC * HD       # 512 features per core
NKT = T // 128       # 16 key tiles
NQC = T // 512       # 4 query chunks (also the x t-chunks)
NCT = C // 128       # 8 contraction tiles
EPS_BIAS = 0.1
SCALE = 1.0 / math.sqrt(HD)


def build(tc, out_ap, xT, wqkv, wproj, ea, tri_dram, dsc1, dsc2):
    """Emit the per-core kernel into TileContext tc.

    out_ap : (T, C)    partial projection output (needs pair-sum on host)
    xT     : (C, T)    x[b] transposed (bf16)
    wqkv   : (C, 3*FPC) [Wq_g | Wk_g | Wv_g] columns for this head group (bf16)
    wproj  : (FPC, C)  W_proj rows for this head group (bf16)
    ea     : (T,)      (prev_probs[b] + 1e-10) ** (-EPS_BIAS)
    tri_dram: (128,128) upper-triangular ones (tri[k,q] = 1 iff k <= q)
    dsc1/dsc2: (16, 1024) DRAM scratch for the denominator shuffle
    """
    nc = tc.nc
    ctx = tc.ctx
    Exp = mybir.ActivationFunctionType.Exp

    const = ctx.enter_context(tc.tile_pool(name="const", bufs=1))
    xs_pool = ctx.enter_context(tc.tile_pool(name="xs", bufs=9))
    qt_pool = ctx.enter_context(tc.tile_pool(name="qt", bufs=6))
    se_pool = ctx.enter_context(tc.tile_pool(name="se", bufs=3))
    tmp_pool = ctx.enter_context(tc.tile_pool(name="tmp", bufs=6))
    rec_pool = ctx.enter_context(tc.tile_pool(name="rec", bufs=3))
    scale_pool = ctx.enter_context(tc.tile_pool(name="scale", bufs=3))
    stack_pool = ctx.enter_context(tc.tile_pool(name="stack", bufs=8))
    pout_pool = ctx.enter_context(tc.tile_pool(name="pout", bufs=3))

    ps_pool = ctx.enter_context(tc.tile_pool(name="ps", bufs=2, space="PSUM"))
    st_pool = ctx.enter_context(tc.tile_pool(name="st", bufs=2, space="PSUM"))
    y_pool = ctx.enter_context(tc.tile_pool(name="y", bufs=2, space="PSUM"))

    # ---- constants / persistent buffers ----
    wq_sb = const.tile([128, NCT, 3 * FPC], BF16, name="wq_sb")     # 24KB/p
    wqkv3 = wqkv.rearrange("(c p) f -> p c f", p=128)
    for c in range(NCT):
        nc.sync.dma_start(out=wq_sb[:, c, :], in_=wqkv3[:, c, :])

    wp_sb = const.tile([128, FPC // 128, C], BF16, name="wp_sb")    # 8KB/p
    nc.sync.dma_start(out=wp_sb, in_=wproj.rearrange("(i p) c -> p i c", p=128))

    kt = const.tile([128, HPC // 2, T], BF16, name="kt")            # 16KB/p
    v2 = const.tile([128, NKT, HPC, HD + 1], BF16, name="v2")       # 16.6KB/p
    eacol = const.tile([128, NKT], F32, name="eacol")
    nc.sync.dma_start(out=eacol, in_=ea.rearrange("(k p) -> p k", p=128))
    ones8 = const.tile([128, HPC], F32, name="ones8")
    nc.vector.memset(ones8, 1.0)
    tri_f32 = const.tile([128, 128], F32, name="tri_f32")
    nc.sync.dma_start(out=tri_f32, in_=tri_dram[:, :])
    tri = const.tile([128, 128], BF16, name="tri")
    nc.vector.tensor_copy(tri, tri_f32)

    # EA columns of v2 (column HD of each head's slot): per-partition scalar
    # broadcast (EA value replicated across the 8 head slots)
    for kt_i in range(NKT):
        nc.vector.tensor_scalar(
            out=v2[:, kt_i, :, HD:HD + 1],
            in0=ones8.unsqueeze(2),
            scalar1=eacol[:, kt_i:kt_i + 1],
            scalar2=None,
            op0=mybir.AluOpType.mult,
        )

    qts_store = {}

    def gen_chunk(qc):
        """Emit one t-chunk's pre-attention work as resumable items, each
        yielding its approximate tensor-engine ns.  Order interleaves Q/K per
        head pair with V tiles so chunk-(qc) attention deps complete early."""
        xs_tiles = []
        for c in range(NCT):
            xs = xs_pool.tile([128, 512], BF16, tag="xs", name=f"xs_{qc}_{c}")
            nc.sync.dma_start(
                out=xs, in_=xT[c * 128:(c + 1) * 128, qc * 512:(qc + 1) * 512]
            )
            xs_tiles.append(xs)
        yield 0

        def q_item(p):
            ps = ps_pool.tile([128, 512], F32, tag="ps", name=f"qps_{qc}_{p}")
            for c in range(NCT):
                nc.tensor.matmul(
                    ps,
                    wq_sb[:, c, p * 128:(p + 1) * 128],
                    xs_tiles[c],
                    start=(c == 0),
                    stop=(c == NCT - 1),
                    skip_group_check=True,
                )
                if c == 3:
                    yield 900
            qt = qt_pool.tile([128, 512], BF16, tag="qt", name=f"qt_{qc}_{p}")
            nc.vector.tensor_copy(qt, ps)
            qts_store.setdefault(qc, {})[p] = qt
            yield 950

        def k_item(p):
            ps = ps_pool.tile([128, 512], F32, tag="ps", name=f"kps_{qc}_{p}")
            for c in range(NCT):
                nc.tensor.matmul(
                    ps,
                    wq_sb[:, c, FPC + p * 128:FPC + (p + 1) * 128],
                    xs_tiles[c],
                    start=(c == 0),
                    stop=(c == NCT - 1),
                    skip_group_check=True,
                )
                if c == 3:
                    yield 900
            nc.vector.tensor_copy(kt[:, p, qc * 512:(qc + 1) * 512], ps)
            yield 950

        def v_item(j):
            kt_i = qc * 4 + j
            ps = ps_pool.tile([128, 512], F32, tag="ps", name=f"vps_{qc}_{j}")
            for c in range(NCT):
                nc.tensor.matmul(
                    ps,
                    xs_tiles[c][:, j * 128:(j + 1) * 128],
                    wq_sb[:, c, 2 * FPC:3 * FPC],
                    start=(c == 0),
                    stop=(c == NCT - 1),
                    skip_group_check=True,
                )
                if c == 3:
                    yield 900
            nc.vector.tensor_scalar(
                out=v2[:, kt_i, :, 0:HD],
                in0=ps.rearrange("p (h d) -> p h d", h=HPC),
                scalar1=eacol[:, kt_i:kt_i + 1],
                scalar2=None,
                op0=mybir.AluOpType.mult,
            )
            yield 950

        order = [q_item(0), k_item(0), v_item(0), v_item(1),
                 q_item(1), k_item(1), v_item(2), v_item(3),
                 q_item(2), k_item(2), q_item(3), k_item(3)]
        for it in order:
            yield from it

    def gen_proj(qc, stacks, pool=None, tag="ps"):
        pool = pool or ps_pool
        for tq in range(4):
            row0 = qc * 512 + tq * 128
            for ch in range(2):
                ps = pool.tile([128, 512], F32, tag=tag,
                               name=f"pps_{qc}_{tq}_{ch}")
                for p in range(HPC // 2):
                    nc.tensor.matmul(
                        ps,
                        stacks[p][:, tq * 128:(tq + 1) * 128],
                        wp_sb[:, p, ch * 512:(ch + 1) * 512],
                        start=(p == 0),
                        stop=(p == HPC // 2 - 1),
                        skip_group_check=True,
                    )
                pout = pout_pool.tile([128, 512], F32, tag="pout",
                                      name=f"po_{qc}_{tq}_{ch}")
                nc.vector.tensor_copy(pout, ps)
                nc.sync.dma_start(
                    out=out_ap[row0:row0 + 128, ch * 512:(ch + 1) * 512],
                    in_=pout,
                )
                yield 1100

    # chunk 0's qkv is generated upfront
    for _ in gen_chunk(0):
        pass

    prev_stacks = None

    for qc in range(NQC):
        fillers = []
        if prev_stacks is not None:
            fillers.append(gen_proj(qc - 1, prev_stacks))
        gen_g = gen_chunk(qc + 1) if qc + 1 < NQC else None
        if gen_g is not None:
            fillers.append(gen_g)
        rr = [0]

        def pull_ns(budget):
            # round-robin the filler generators until ~budget ns of tensor
            # work has been emitted
            while budget > 0 and fillers:
                g = fillers[rr[0] % len(fillers)]
                rr[0] += 1
                try:
                    est = next(g)
                    budget -= est if est is not None else 600
                except StopIteration:
                    fillers.remove(g)

        nki = 4 * qc + 4
        stacks = []
        for p in range(HPC // 2):
            qt = qts_store[qc][p]
            yA = y_pool.tile([128, 512], F32, tag="y", name=f"yA_{qc}_{p}")
            yB = y_pool.tile([128, 512], F32, tag="y", name=f"yB_{qc}_{p}")

            def n0_of(k):
                r = k - 4 * qc
                return 128 * r if r > 0 else 0

            ses = {}

            def emit_pv(k):
                n0p = n0_of(k)
                se3p = ses[k]
                nc.tensor.matmul(
                    yA[0:HD + 1, n0p:512], v2[:, k, 2 * p, :],
                    se3p[:, 0, n0p:512],
                    start=(k == 0), stop=(k == nki - 1), skip_group_check=True,
                )
                nc.tensor.matmul(
                    yB[0:HD + 1, n0p:512], v2[:, k, 2 * p + 1, :],
                    se3p[:, 1, n0p:512],
                    start=(k == 0), stop=(k == nki - 1), skip_group_check=True,
                )
                return 2 * ((512 - n0p) / 2.4 + 20)

            for ki in range(nki):
                r = ki - 4 * qc
                n0 = n0_of(ki)
                st = st_pool.tile([128, 1024], F32, tag="st",
                                  name=f"st_{qc}_{p}_{ki}")
                st3 = st.rearrange("p (h q) -> p h q", h=2)
                kslice = slice(ki * 128, (ki + 1) * 128)
                nc.tensor.matmul(
                    st3[:, 0, n0:512], kt[0:64, p, kslice], qt[0:64, n0:512],
                    start=True, stop=True,
                )
                nc.tensor.matmul(
                    st3[:, 1, n0:512], kt[64:128, p, kslice], qt[64:128, n0:512],
                    start=True, stop=True,
                )
                se = se_pool.tile([128, 1024], BF16, tag="se",
                                  name=f"se_{qc}_{p}_{ki}")
                se3 = se.rearrange("p (h q) -> p h q", h=2)
                nc.scalar.activation(
                    se3[:, :, n0:512], st3[:, :, n0:512], Exp, scale=SCALE
                )
                if r >= 0:
                    nc.vector.tensor_mul(
                        se3[:, 0, n0:n0 + 128], se3[:, 0, n0:n0 + 128], tri
                    )
                    nc.vector.tensor_mul(
                        se3[:, 1, n0:n0 + 128], se3[:, 1, n0:n0 + 128], tri
                    )
                ses[ki] = se3

                tensor_ns = (512 - n0) / 2.4 + 25
                if ki >= 2:
                    tensor_ns += emit_pv(ki - 2)
                exp_ns = (2 * (512 - n0) + 352) / 1.2
                chain_extra = 450 if r >= 0 else 0
                pull_ns(exp_ns + chain_extra - tensor_ns)

            emit_pv(nki - 2)
            pull_ns(500)
            emit_pv(nki - 1)

            # evacuate y^T + denominators (DVE — scalar stays pure exp)
            tmpA = tmp_pool.tile([128, 512], F32, tag="tmp", name=f"tmpA_{qc}_{p}")
            nc.vector.tensor_copy(tmpA[0:HD + 1, :], yA[0:HD + 1, :])
            tmpB = tmp_pool.tile([128, 512], F32, tag="tmp", name=f"tmpB_{qc}_{p}")
            nc.vector.tensor_copy(tmpB[0:HD + 1, :], yB[0:HD + 1, :])

            # 1/denominator: bounce rows through DRAM to spread the 1024
            # values over 128 partitions (reciprocal runs at 8 cyc/elem on
            # the free dim), then broadcast straight from DRAM.  All hops
            # ride the otherwise-idle gpsimd SWDGE queue; deep tmp/sc pools
            # keep the latency off every critical chain except the tail.
            idx = qc * 4 + p
            nc.gpsimd.dma_start(out=dsc1[idx, 0:512], in_=tmpA[HD:HD + 1, :])
            nc.gpsimd.dma_start(out=dsc1[idx, 512:1024], in_=tmpB[HD:HD + 1, :])
            dnp = rec_pool.tile([128, 8], F32, tag="dnp", name=f"dnp_{qc}_{p}")
            nc.gpsimd.dma_start(
                out=dnp, in_=dsc1[idx, :].rearrange("(j p) -> p j", p=128)
            )
            rcp = rec_pool.tile([128, 8], F32, tag="rcp", name=f"rcp_{qc}_{p}")
            nc.vector.reciprocal(rcp, dnp)
            nc.gpsimd.dma_start(
                out=dsc2[idx, :].rearrange("(j p) -> p j", p=128), in_=rcp
            )
            sc = scale_pool.tile([64, 1024], F32, tag="sc", name=f"sc_{qc}_{p}")
            nc.gpsimd.dma_start(
                out=sc[0:64, :], in_=dsc2[idx:idx + 1, :].to_broadcast([64, 1024])
            )

            stack = stack_pool.tile([128, 512], BF16, tag="stack",
                                    name=f"stk_{qc}_{p}")
            nc.vector.tensor_mul(stack[0:64, :], tmpA[0:64, :], sc[0:64, 0:512])
            stkB = tmp_pool.tile([64, 512], BF16, tag="stkB", bufs=3,
                                 name=f"skB_{qc}_{p}")
            nc.vector.tensor_mul(stkB[0:64, :], tmpB[0:64, :], sc[0:64, 512:1024])
            nc.sync.dma_start(out=stack[64:128, :], in_=stkB[0:64, :])
            stacks.append(stack)
            pull_ns(1500)

        # invariant: gen(qc+1) fully emitted before qc+1's attention
        pull_ns(10_000_000)
        prev_stacks = stacks

    for _ in gen_proj(NQC - 1, prev_stacks, pool=y_pool, tag="y"):
        pass


def make_nc():
    nc = bacc.Bacc("TRN2", target_bir_lowering=False, debug=False,
                   num_devices=NCORES)
    xT = nc.dram_tensor("xT", [C, T], BF16, kind="ExternalInput")
    wqkv = nc.dram_tensor("wqkv", [C, 3 * FPC], BF16, kind="ExternalInput")
    wproj = nc.dram_tensor("wproj", [FPC, C], BF16, kind="ExternalInput")
    ea = nc.dram_tensor("ea", [T], F32, kind="ExternalInput")
    out = nc.dram_tensor("out", [T, C], F32, kind="ExternalOutput")
    dsc1 = nc.dram_tensor("dsc1", [16, 1024], F32, kind="Internal")
    dsc2 = nc.dram_tensor("dsc2", [16, 1024], F32, kind="Internal")
    tri_np = np.triu(np.ones((128, 128), dtype=np.float32))
    tri_dram = nc.inline_tensor(tri_np, name="tri_const")
    with ExitStack() as ctx:
        tc = ctx.enter_context(tile.TileContext(nc))
        tc.ctx = ctx
        build(tc, out[:, :], xT[:, :], wqkv[:, :], wproj[:, :], ea[:],
              tri_dram, dsc1[:, :], dsc2[:, :])
    nc.compile()
    return nc


def shard_inputs(x, prev_probs, W_attn, W_proj):
    import ml_dtypes

    bf16 = ml_dtypes.bfloat16
    in_maps = []
    for core in range(NCORES):
        b, g = divmod(core, 2)
        xT = np.ascontiguousarray(x[b].T)
        wq = W_attn[:, g * FPC:(g + 1) * FPC]
        wk = W_attn[:, C + g * FPC:C + (g + 1) * FPC]
        wv = W_attn[:, 2 * C + g * FPC:2 * C + (g + 1) * FPC]
        wqkv = np.ascontiguousarray(np.concatenate([wq, wk, wv], axis=1))
        wproj = np.ascontiguousarray(W_proj[g * FPC:(g + 1) * FPC, :])
        ea = np.power(prev_probs[b] + np.float32(1e-10), np.float32(-EPS_BIAS))
        in_maps.append(
            {
                "xT": xT.astype(bf16),
                "wqkv": wqkv.astype(bf16),
                "wproj": wproj.astype(bf16),
                "ea": ea.astype(np.float32),
            }
        )
    return in_maps


_CACHED_NC = None


def kernel(x, prev_probs, W_attn, W_proj, trace=False, tmpdir=None):
    global _CACHED_NC
    from concourse.bass_utils import run_bass_kernel_spmd

    x = np.asarray(x, dtype=np.float32)
    prev_probs = np.asarray(prev_probs, dtype=np.float32)
    W_attn = np.asarray(W_attn, dtype=np.float32)
    W_proj = np.asarray(W_proj, dtype=np.float32)

    if _CACHED_NC is None:
        _CACHED_NC = make_nc()
    nc = _CACHED_NC

    in_maps = shard_inputs(x, prev_probs, W_attn, W_proj)
    res = run_bass_kernel_spmd(
        nc, in_maps, core_ids=list(range(NCORES)), trace=trace, tmpdir=tmpdir
    )
    parts = [r["out"] for r in res.results]
    out = np.empty((B, T, C), dtype=np.float32)
    for b in range(B):
        out[b] = parts[2 * b] + parts[2 * b + 1]
    kernel.last_results = res
    return out


# revision 5
# speedup vs baseline: 1.2737x; 1.2737x over previous
"""Trainium2 Bass kernel for CausalSelfAttention (B=4, T=2048, C=1024, H=16)
with additive prev-prob key bias.

Sharding: 8 cores = data-parallel over B (4) x tensor-parallel over head
halves (2).  Each core computes qkv for its 8 heads, causal attention, and a
partial output projection (row-parallel W_proj); host sums the two partials
per batch at unshard time.

Per-core device algorithm (v3 — gap-free tensor queue, decoupled engines):
  - QKV/proj matmuls run bf16 (host-cast inputs; FWL weight loads, halved
    DMA); PV runs fp32r (se produced by ACT at full fp32r exp rate).  PSUM
    accumulation is always fp32 and scores enter exp at fp32.
  - K^T and Q^T feature-major with head pairs stacked in the 128 partitions;
    QK^T is a row-tiled concurrent pair (K=64 at base partitions 0/64).
  - Scores transposed (keys on partitions); softmax denominator comes from an
    EA column appended to V (M=65 PV matmuls), EA[k] = (p[k]+1e-10)**-EPS.
  - Causality: block-trimmed matmul widths; the diagonal 128x128 mask is a
    PSUM seed (identity-matmul writes -1e30 above the diagonal, QK then
    accumulates start=False) so the QK->exp->PV chain crosses only
    tensor/scalar.  No mask work on DVE.
  - The scalar (ACT) engine runs a pure exp stream; PV is software-pipelined
    two score-tiles behind QK; QKV-gen of the next chunk and projection of
    the previous chunk fill remaining tensor slack via a deficit-driven pull.
  - Per-(head,query) 1/denominator: DRAM partition-scatter bounce on the
    gpsimd SWDGE queue, with the whole chain (spread read, approx
    reciprocal, broadcast, normalize multiplies) deferred into the NEXT
    pair's slot stream at one hop per slot — no in-order queue ever waits
    on a DMA round trip.
"""

import math
from contextlib import ExitStack

import numpy as np

import concourse.bass as bass
import concourse.tile as tile
from concourse import bacc, mybir

F32 = mybir.dt.float32
F32R = mybir.dt.float32r
BF16 = mybir.dt.bfloat16

B, T, C, H = 4, 2048, 1024, 16
HD = C // H          # 64
NCORES = 8
HPC = H // 2         # 8 heads per core
FPC = HPC * HD       # 512 features per core
NKT = T // 128       # 16 key tiles
NQC = T // 512       # 4 query chunks (also the x t-chunks)
NCT = C // 128       # 8 contraction tiles
EPS_BIAS = 0.1
SCALE = 1.0 / math.sqrt(HD)
NEG = -1.0e30


def build(tc, out_ap, xT, wqkv, wproj, ea, trineg_dram, id_dram, dsc1, dsc2):
    """Emit the per-core kernel into TileContext tc.

    out_ap : (T, C)    partial projection output (needs pair-sum on host)
    xT     : (C, T)    x[b] transposed (bf16)
    wqkv   : (C, 3*FPC) [Wq_g | Wk_g | Wv_g] columns for this head group (bf16)
    wproj  : (FPC, C)  W_proj rows for this head group (bf16)
    ea     : (T,)      (prev_probs[b] + 1e-10) ** (-EPS_BIAS)
    trineg_dram: (128,128) 0 on/below diagonal (k<=q), -1e30 above
    id_dram: (128,128) identity
    dsc1/dsc2: (16, 1024) DRAM scratch for the denominator shuffle
    """
    nc = tc.nc
    ctx = tc.ctx
    Exp = mybir.ActivationFunctionType.Exp

    const = ctx.enter_context(tc.tile_pool(name="const", bufs=1))
    xs_pool = ctx.enter_context(tc.tile_pool(name="xs", bufs=9))
    qt_pool = ctx.enter_context(tc.tile_pool(name="qt", bufs=6))
    se_pool = ctx.enter_context(tc.tile_pool(name="se", bufs=3))
    tmp_pool = ctx.enter_context(tc.tile_pool(name="tmp", bufs=6))
    rec_pool = ctx.enter_context(tc.tile_pool(name="rec", bufs=3))
    scale_pool = ctx.enter_context(tc.tile_pool(name="scale", bufs=3))
    stack_pool = ctx.enter_context(tc.tile_pool(name="stack", bufs=8))
    pout_pool = ctx.enter_context(tc.tile_pool(name="pout", bufs=3))

    ps_pool = ctx.enter_context(tc.tile_pool(name="ps", bufs=2, space="PSUM"))
    st_pool = ctx.enter_context(tc.tile_pool(name="st", bufs=2, space="PSUM"))
    y_pool = ctx.enter_context(tc.tile_pool(name="y", bufs=2, space="PSUM"))

    # ---- constants / persistent buffers ----
    wq_sb = const.tile([128, NCT, 3 * FPC], BF16, name="wq_sb")     # 24KB/p
    wqkv3 = wqkv.rearrange("(c p) f -> p c f", p=128)
    for c in range(NCT):
        nc.sync.dma_start(out=wq_sb[:, c, :], in_=wqkv3[:, c, :])

    wp_sb = const.tile([128, FPC // 128, C], BF16, name="wp_sb")    # 8KB/p
    nc.sync.dma_start(out=wp_sb, in_=wproj.rearrange("(i p) c -> p i c", p=128))

    kt = const.tile([128, HPC // 2, T], BF16, name="kt")            # 16KB/p
    v2 = const.tile([128, NKT, HPC, HD + 1], F32R, name="v2")       # 33.3KB/p
    eacol = const.tile([128, NKT], F32, name="eacol")
    nc.sync.dma_start(out=eacol, in_=ea.rearrange("(k p) -> p k", p=128))
    ones8 = const.tile([128, HPC], F32, name="ones8")
    nc.vector.memset(ones8, 1.0)
    tn_f32 = const.tile([128, 128], F32, name="tn_f32")
    nc.sync.dma_start(out=tn_f32, in_=trineg_dram[:, :])
    trineg = const.tile([128, 128], BF16, name="trineg")
    nc.vector.tensor_copy(trineg, tn_f32)
    id_f32 = const.tile([128, 128], F32, name="id_f32")
    nc.sync.dma_start(out=id_f32, in_=id_dram[:, :])
    id128 = const.tile([128, 128], BF16, name="id128")
    nc.vector.tensor_copy(id128, id_f32)

    # EA columns of v2 (column HD of each head's slot)
    for kt_i in range(NKT):
        nc.vector.tensor_scalar(
            out=v2[:, kt_i, :, HD:HD + 1],
            in0=ones8.unsqueeze(2),
            scalar1=eacol[:, kt_i:kt_i + 1],
            scalar2=None,
            op0=mybir.AluOpType.mult,
        )

    qts_store = {}
    stacks_store = {}

    def gen_chunk(qc):
        """One t-chunk's pre-attention work as resumable items, each yielding
        its approximate tensor-engine ns."""
        xs_tiles = []
        for c in range(NCT):
            xs = xs_pool.tile([128, 512], BF16, tag="xs", name=f"xs_{qc}_{c}")
            nc.sync.dma_start(
                out=xs, in_=xT[c * 128:(c + 1) * 128, qc * 512:(qc + 1) * 512]
            )
            xs_tiles.append(xs)
        yield 0

        def q_item(p):
            ps = ps_pool.tile([128, 512], F32, tag="ps", name=f"qps_{qc}_{p}")
            for c in range(NCT):
                nc.tensor.matmul(
                    ps,
                    wq_sb[:, c, p * 128:(p + 1) * 128],
                    xs_tiles[c],
                    start=(c == 0),
                    stop=(c == NCT - 1),
                    skip_group_check=True,
                )
                if c == 3:
                    yield 900
            qt = qt_pool.tile([128, 512], BF16, tag="qt", name=f"qt_{qc}_{p}")
            nc.vector.tensor_copy(qt, ps)
            qts_store.setdefault(qc, {})[p] = qt
            yield 950

        def k_item(p):
            ps = ps_pool.tile([128, 512], F32, tag="ps", name=f"kps_{qc}_{p}")
            for c in range(NCT):
                nc.tensor.matmul(
                    ps,
                    wq_sb[:, c, FPC + p * 128:FPC + (p + 1) * 128],
                    xs_tiles[c],
                    start=(c == 0),
                    stop=(c == NCT - 1),
                    skip_group_check=True,
                )
                if c == 3:
                    yield 900
            nc.vector.tensor_copy(kt[:, p, qc * 512:(qc + 1) * 512], ps)
            yield 950

        def v_item(j):
            kt_i = qc * 4 + j
            ps = ps_pool.tile([128, 512], F32, tag="ps", name=f"vps_{qc}_{j}")
            for c in range(NCT):
                nc.tensor.matmul(
                    ps,
                    xs_tiles[c][:, j * 128:(j + 1) * 128],
                    wq_sb[:, c, 2 * FPC:3 * FPC],
                    start=(c == 0),
                    stop=(c == NCT - 1),
                    skip_group_check=True,
                )
                if c == 3:
                    yield 900
            nc.vector.tensor_scalar(
                out=v2[:, kt_i, :, 0:HD],
                in0=ps.rearrange("p (h d) -> p h d", h=HPC),
                scalar1=eacol[:, kt_i:kt_i + 1],
                scalar2=None,
                op0=mybir.AluOpType.mult,
            )
            yield 950

        order = [q_item(0), k_item(0), v_item(0), v_item(1),
                 q_item(1), k_item(1), v_item(2), v_item(3),
                 q_item(2), k_item(2), q_item(3), k_item(3)]
        for it in order:
            yield from it

    def den_chain(qc, p, tmpA, tmpB):
        """Deferred denominator chain for pair (qc,p): spread-read, approx
        reciprocal, broadcast, normalize multiplies, stack assembly.  Pulled
        ~one item per slot during the NEXT pair so no queue waits on a DMA
        round trip."""
        idx = qc * 4 + p
        dnp = rec_pool.tile([128, 8], F32, tag="dnp", name=f"dnp_{qc}_{p}")
        nc.gpsimd.dma_start(
            out=dnp, in_=dsc1[idx, :].rearrange("(j p) -> p j", p=128)
        )
        yield 0
        yield 0
        rcp = rec_pool.tile([128, 8], F32, tag="rcp", name=f"rcp_{qc}_{p}")
        nc.vector.reciprocal_approx_fast(out=rcp, in_=dnp)
        yield 0
        nc.gpsimd.dma_start(
            out=dsc2[idx, :].rearrange("(j p) -> p j", p=128), in_=rcp
        )
        yield 0
        sc = scale_pool.tile([64, 1024], F32, tag="sc", name=f"sc_{qc}_{p}")
        nc.gpsimd.dma_start(
            out=sc[0:64, :], in_=dsc2[idx:idx + 1, :].to_broadcast([64, 1024])
        )
        yield 0
        yield 0
        yield 0
        stack = stack_pool.tile([128, 512], BF16, tag="stack",
                                name=f"stk_{qc}_{p}")
        nc.vector.tensor_mul(stack[0:64, :], tmpA[0:64, :], sc[0:64, 0:512])
        yield 100
        stkB = tmp_pool.tile([64, 512], BF16, tag="stkB", bufs=3,
                             name=f"skB_{qc}_{p}")
        nc.vector.tensor_mul(stkB[0:64, :], tmpB[0:64, :], sc[0:64, 512:1024])
        nc.sync.dma_start(out=stack[64:128, :], in_=stkB[0:64, :])
        stacks_store.setdefault(qc, {})[p] = stack
        yield 100

    def gen_proj(qc):
        stacks = stacks_store[qc]
        pool = y_pool if qc == NQC - 1 else ps_pool
        tag = "y" if qc == NQC - 1 else "ps"
        for tq in range(4):
            row0 = qc * 512 + tq * 128
            for ch in range(2):
                ps = pool.tile([128, 512], F32, tag=tag,
                               name=f"pps_{qc}_{tq}_{ch}")
                for p in range(HPC // 2):
                    nc.tensor.matmul(
                        ps,
                        stacks[p][:, tq * 128:(tq + 1) * 128],
                        wp_sb[:, p, ch * 512:(ch + 1) * 512],
                        start=(p == 0),
                        stop=(p == HPC // 2 - 1),
                        skip_group_check=True,
                    )
                pout = pout_pool.tile([128, 512], F32, tag="pout",
                                      name=f"po_{qc}_{tq}_{ch}")
                nc.vector.tensor_copy(pout, ps)
                nc.sync.dma_start(
                    out=out_ap[row0:row0 + 128, ch * 512:(ch + 1) * 512],
                    in_=pout,
                )
                yield 1100

    # chunk 0's qkv generated upfront
    for _ in gen_chunk(0):
        pass

    deferred = []      # den chains: paced at most one item per pull call
    mains = []         # gen / proj generators: budget-filled
    proj_pending = []  # qc values whose proj hasn't been queued yet

    def pull_ns(budget):
        # 1. at most one deferred (den-chain) item per call
        if deferred:
            try:
                budget -= next(deferred[0]) or 0
            except StopIteration:
                deferred.pop(0)
        # 2. unlock proj(qc') once its stacks are all written
        if proj_pending and len(stacks_store.get(proj_pending[0], {})) == 4:
            mains.append(gen_proj(proj_pending.pop(0)))
        # 3. budget-fill from main generators, in order
        while budget > 0 and mains:
            try:
                est = next(mains[0])
                budget -= est if est is not None else 600
            except StopIteration:
                mains.pop(0)
                if (proj_pending
                        and len(stacks_store.get(proj_pending[0], {})) == 4):
                    mains.append(gen_proj(proj_pending.pop(0)))

    for qc in range(NQC):
        if qc + 1 < NQC:
            mains.append(gen_chunk(qc + 1))
        if qc > 0:
            proj_pending.append(qc - 1)

        nki = 4 * qc + 4
        for p in range(HPC // 2):
            qt = qts_store[qc][p]
            yA = y_pool.tile([128, 512], F32, tag="y", name=f"yA_{qc}_{p}")
            yB = y_pool.tile([128, 512], F32, tag="y", name=f"yB_{qc}_{p}")

            def n0_of(k):
                r = k - 4 * qc
                return 128 * r if r > 0 else 0

            ses = {}

            def emit_pv(k):
                n0p = n0_of(k)
                se3p = ses.pop(k)
                nc.tensor.matmul(
                    yA[0:HD + 1, n0p:512], v2[:, k, 2 * p, :],
                    se3p[:, 0, n0p:512],
                    start=(k == 0), stop=(k == nki - 1), skip_group_check=True,
                )
                nc.tensor.matmul(
                    yB[0:HD + 1, n0p:512], v2[:, k, 2 * p + 1, :],
                    se3p[:, 1, n0p:512],
                    start=(k == 0), stop=(k == nki - 1), skip_group_check=True,
                )
                return 2 * ((512 - n0p) / 2.4 + 20)

            for ki in range(nki):
                r = ki - 4 * qc
                n0 = n0_of(ki)
                st = st_pool.tile([128, 1024], F32, tag="st",
                                  name=f"st_{qc}_{p}_{ki}")
                st3 = st.rearrange("p (h q) -> p h q", h=2)
                kslice = slice(ki * 128, (ki + 1) * 128)
                diag = r >= 0
                if diag:
                    # causal mask: seed the diagonal block with -1e30 above
                    # the diagonal (identity matmul); QK accumulates on top
                    nc.tensor.matmul(
                        st3[:, 0, n0:n0 + 128], id128, trineg,
                        start=True, stop=False, skip_group_check=True,
                    )
                    nc.tensor.matmul(
                        st3[:, 1, n0:n0 + 128], id128, trineg,
                        start=True, stop=False, skip_group_check=True,
                    )
                nc.tensor.matmul(
                    st3[:, 0, n0:512], kt[0:64, p, kslice], qt[0:64, n0:512],
                    start=not diag, stop=True, skip_group_check=True,
                )
                nc.tensor.matmul(
                    st3[:, 1, n0:512], kt[64:128, p, kslice], qt[64:128, n0:512],
                    start=not diag, stop=True, skip_group_check=True,
                )
                se = se_pool.tile([128, 1024], F32R, tag="se",
                                  name=f"se_{qc}_{p}_{ki}")
                se3 = se.rearrange("p (h q) -> p h q", h=2)
                nc.scalar.activation(
                    se3[:, :, n0:512], st3[:, :, n0:512], Exp, scale=SCALE
                )
                ses[ki] = se3

                tensor_ns = (512 - n0) / 2.4 + 25 + (110 if diag else 0)
                if ki >= 2:
                    tensor_ns += emit_pv(ki - 2)
                exp_ns = (2 * (512 - n0) + 352) / 1.2
                pull_ns(exp_ns - tensor_ns)

            emit_pv(nki - 2)
            pull_ns(500)
            emit_pv(nki - 1)

            # evacuate y^T + denominator rows (DVE; scalar stays pure exp)
            tmpA = tmp_pool.tile([128, 512], F32, tag="tmp", name=f"tmpA_{qc}_{p}")
            nc.vector.tensor_copy(tmpA[0:HD + 1, :], yA[0:HD + 1, :])
            tmpB = tmp_pool.tile([128, 512], F32, tag="tmp", name=f"tmpB_{qc}_{p}")
            nc.vector.tensor_copy(tmpB[0:HD + 1, :], yB[0:HD + 1, :])
            idx = qc * 4 + p
            nc.gpsimd.dma_start(out=dsc1[idx, 0:512], in_=tmpA[HD:HD + 1, :])
            nc.gpsimd.dma_start(out=dsc1[idx, 512:1024], in_=tmpB[HD:HD + 1, :])
            deferred.append(den_chain(qc, p, tmpA, tmpB))
            pull_ns(1200)

        # invariant: gen(qc+1) fully emitted before qc+1's attention
        # (paced so deferred den chains keep advancing through the dump)
        while mains:
            pull_ns(2000)

    # tail: drain remaining den chains back-to-back, then final projections
    while deferred:
        pull_ns(0)
    while proj_pending:
        qcq = proj_pending.pop(0)
        for _ in gen_proj(qcq):
            pass
    for _ in gen_proj(NQC - 1):
        pass


def make_nc():
    nc = bacc.Bacc("TRN2", target_bir_lowering=False, debug=False,
                   num_devices=NCORES)
    xT = nc.dram_tensor("xT", [C, T], BF16, kind="ExternalInput")
    wqkv = nc.dram_tensor("wqkv", [C, 3 * FPC], BF16, kind="ExternalInput")
    wproj = nc.dram_tensor("wproj", [FPC, C], BF16, kind="ExternalInput")
    ea = nc.dram_tensor("ea", [T], F32, kind="ExternalInput")
    out = nc.dram_tensor("out", [T, C], F32, kind="ExternalOutput")
    dsc1 = nc.dram_tensor("dsc1", [16, 1024], F32, kind="Internal")
    dsc2 = nc.dram_tensor("dsc2", [16, 1024], F32, kind="Internal")
    kq = np.arange(128)
    trineg_np = np.where(kq[:, None] <= kq[None, :], 0.0, NEG).astype(np.float32)
    trineg_dram = nc.inline_tensor(trineg_np, name="trineg_const")
    id_np = np.eye(128, dtype=np.float32)
    id_dram = nc.inline_tensor(id_np, name="id_const")
    with ExitStack() as ctx:
        tc = ctx.enter_context(tile.TileContext(nc))
        tc.ctx = ctx
        build(tc, out[:, :], xT[:, :], wqkv[:, :], wproj[:, :], ea[:],
              trineg_dram, id_dram, dsc1[:, :], dsc2[:, :])
    nc.compile()
    return nc


def shard_inputs(x, prev_probs, W_attn, W_proj):
    import ml_dtypes

    bf16 = ml_dtypes.bfloat16
    in_maps = []
    for core in range(NCORES):
        b, g = divmod(core, 2)
        xT = np.ascontiguousarray(x[b].T)
        wq = W_attn[:, g * FPC:(g + 1) * FPC]
        wk = W_attn[:, C + g * FPC:C + (g + 1) * FPC]
        wv = W_attn[:, 2 * C + g * FPC:2 * C + (g + 1) * FPC]
        wqkv = np.ascontiguousarray(np.concatenate([wq, wk, wv], axis=1))
        wproj = np.ascontiguousarray(W_proj[g * FPC:(g + 1) * FPC, :])
        ea = np.power(prev_probs[b] + np.float32(1e-10), np.float32(-EPS_BIAS))
        in_maps.append(
            {
                "xT": xT.astype(bf16),
                "wqkv": wqkv.astype(bf16),
                "wproj": wproj.astype(bf16),
                "ea": ea.astype(np.float32),
            }
        )
    return in_maps


_CACHED_NC = None


def kernel(x, prev_probs, W_attn, W_proj, trace=False, tmpdir=None):
    global _CACHED_NC
    from concourse.bass_utils import run_bass_kernel_spmd

    x = np.asarray(x, dtype=np.float32)
    prev_probs = np.asarray(prev_probs, dtype=np.float32)
    W_attn = np.asarray(W_attn, dtype=np.float32)
    W_proj = np.asarray(W_proj, dtype=np.float32)

    if _CACHED_NC is None:
        _CACHED_NC = make_nc()
    nc = _CACHED_NC

    in_maps = shard_inputs(x, prev_probs, W_attn, W_proj)
    res = run_bass_kernel_spmd(
        nc, in_maps, core_ids=list(range(NCORES)), trace=trace, tmpdir=tmpdir
    )
    parts = [r["out"] for r in res.results]
    out = np.empty((B, T, C), dtype=np.float32)
    for b in range(B):
        out[b] = parts[2 * b] + parts[2 * b + 1]
    kernel.last_results = res
    return out


# revision 9
# speedup vs baseline: 1.3080x; 1.0269x over previous
"""Trainium2 Bass kernel for CausalSelfAttention (B=4, T=2048, C=1024, H=16)
with additive prev-prob key bias.

Sharding: 8 cores = data-parallel over B (4) x tensor-parallel over head
halves (2).  Each core computes qkv for its 8 heads, causal attention, and a
partial output projection (row-parallel W_proj); host sums the two partials
per batch at unshard time.

Per-core device algorithm (v3 — gap-free tensor queue, decoupled engines):
  - QKV/proj matmuls run bf16 (host-cast inputs; FWL weight loads, halved
    DMA); PV runs fp32r (se produced by ACT at full fp32r exp rate).  PSUM
    accumulation is always fp32 and scores enter exp at fp32.
  - K^T and Q^T feature-major with head pairs stacked in the 128 partitions;
    QK^T is a row-tiled concurrent pair (K=64 at base partitions 0/64).
  - Scores transposed (keys on partitions); softmax denominator comes from an
    EA column appended to V (M=65 PV matmuls), EA[k] = (p[k]+1e-10)**-EPS.
  - Causality: block-trimmed matmul widths; the diagonal 128x128 mask is a
    PSUM seed (identity-matmul writes -1e30 above the diagonal, QK then
    accumulates start=False) so the QK->exp->PV chain crosses only
    tensor/scalar.  No mask work on DVE.
  - The scalar (ACT) engine runs a pure exp stream; PV is software-pipelined
    two score-tiles behind QK; QKV-gen of the next chunk and projection of
    the previous chunk fill remaining tensor slack via a deficit-driven pull.
  - Per-(head,query) 1/denominator: DRAM partition-scatter bounce on the
    gpsimd SWDGE queue, with the whole chain (spread read, approx
    reciprocal, broadcast, normalize multiplies) deferred into the NEXT
    pair's slot stream at one hop per slot — no in-order queue ever waits
    on a DMA round trip.
"""

import math
from contextlib import ExitStack

import numpy as np

import concourse.bass as bass
import concourse.tile as tile
from concourse import bacc, mybir

F32 = mybir.dt.float32
F32R = mybir.dt.float32r
BF16 = mybir.dt.bfloat16

B, T, C, H = 4, 2048, 1024, 16
HD = C // H          # 64
NCORES = 8
HPC = H // 2         # 8 heads per core
FPC = HPC * HD       # 512 features per core
NKT = T // 128       # 16 key tiles
NQC = T // 512       # 4 query chunks (also the x t-chunks)
NCT = C // 128       # 8 contraction tiles
EPS_BIAS = 0.1
SCALE = 1.0 / math.sqrt(HD)
NEG = -1.0e30


def build(tc, out_ap, xT, wqkv, wproj, ea, trineg_dram, id_dram, dsc1, dsc2):
    """Emit the per-core kernel into TileContext tc.

    out_ap : (T, C)    partial projection output (needs pair-sum on host)
    xT     : (C, T)    x[b] transposed (bf16)
    wqkv   : (C, 3*FPC) [Wq_g | Wk_g | Wv_g] columns for this head group (bf16)
    wproj  : (FPC, C)  W_proj rows for this head group (bf16)
    ea     : (T,)      (prev_probs[b] + 1e-10) ** (-EPS_BIAS)
    trineg_dram: (128,128) 0 on/below diagonal (k<=q), -1e30 above
    id_dram: (128,128) identity
    dsc1/dsc2: (16, 1024) DRAM scratch for the denominator shuffle
    """
    nc = tc.nc
    ctx = tc.ctx
    Exp = mybir.ActivationFunctionType.Exp

    const = ctx.enter_context(tc.tile_pool(name="const", bufs=1))
    xs_pool = ctx.enter_context(tc.tile_pool(name="xs", bufs=9))
    qt_pool = ctx.enter_context(tc.tile_pool(name="qt", bufs=6))
    se_pool = ctx.enter_context(tc.tile_pool(name="se", bufs=3))
    tmp_pool = ctx.enter_context(tc.tile_pool(name="tmp", bufs=8))
    rec_pool = ctx.enter_context(tc.tile_pool(name="rec", bufs=4))
    scale_pool = ctx.enter_context(tc.tile_pool(name="scale", bufs=4))
    stack_pool = ctx.enter_context(tc.tile_pool(name="stack", bufs=8))
    pout_pool = ctx.enter_context(tc.tile_pool(name="pout", bufs=3))

    ps_pool = ctx.enter_context(tc.tile_pool(name="ps", bufs=2, space="PSUM"))
    st_pool = ctx.enter_context(tc.tile_pool(name="st", bufs=2, space="PSUM"))
    y_pool = ctx.enter_context(tc.tile_pool(name="y", bufs=2, space="PSUM"))

    # ---- constants / persistent buffers ----
    wq_sb = const.tile([128, NCT, 3 * FPC], BF16, name="wq_sb")     # 24KB/p
    wqkv3 = wqkv.rearrange("(c p) f -> p c f", p=128)
    for c in range(NCT):
        nc.sync.dma_start(out=wq_sb[:, c, :], in_=wqkv3[:, c, :])

    wp_sb = const.tile([128, FPC // 128, C], BF16, name="wp_sb")    # 8KB/p
    nc.sync.dma_start(out=wp_sb, in_=wproj.rearrange("(i p) c -> p i c", p=128))

    kt = const.tile([128, HPC // 2, T], BF16, name="kt")            # 16KB/p
    v2 = const.tile([128, NKT, HPC, HD + 1], F32R, name="v2")       # 33.3KB/p
    eacol = const.tile([128, NKT], F32, name="eacol")
    nc.sync.dma_start(out=eacol, in_=ea.rearrange("(k p) -> p k", p=128))
    ones8 = const.tile([128, HPC], F32, name="ones8")
    nc.vector.memset(ones8, 1.0)
    tn_f32 = const.tile([128, 128], F32, name="tn_f32")
    nc.sync.dma_start(out=tn_f32, in_=trineg_dram[:, :])
    trineg = const.tile([128, 128], BF16, name="trineg")
    nc.vector.tensor_copy(trineg, tn_f32)
    id_f32 = const.tile([128, 128], F32, name="id_f32")
    nc.sync.dma_start(out=id_f32, in_=id_dram[:, :])
    id128 = const.tile([128, 128], BF16, name="id128")
    nc.vector.tensor_copy(id128, id_f32)

    # EA columns of v2 (column HD of each head's slot)
    for kt_i in range(NKT):
        nc.vector.tensor_scalar(
            out=v2[:, kt_i, :, HD:HD + 1],
            in0=ones8.unsqueeze(2),
            scalar1=eacol[:, kt_i:kt_i + 1],
            scalar2=None,
            op0=mybir.AluOpType.mult,
        )

    qts_store = {}
    stacks_store = {}
    xs_store = {}

    def load_xs(qc):
        xs_tiles = []
        for c in range(NCT):
            xs = xs_pool.tile([128, 512], BF16, tag="xs", name=f"xs_{qc}_{c}")
            nc.sync.dma_start(
                out=xs, in_=xT[c * 128:(c + 1) * 128, qc * 512:(qc + 1) * 512]
            )
            xs_tiles.append(xs)
        xs_store[qc] = xs_tiles

    def qk_item(qc, p):
        """Q^T then K^T for head pair p of chunk qc."""
        xs_tiles = xs_store[qc]
        ps = ps_pool.tile([128, 512], F32, tag="ps", name=f"qps_{qc}_{p}")
        for c in range(NCT):
            nc.tensor.matmul(
                ps,
                wq_sb[:, c, p * 128:(p + 1) * 128],
                xs_tiles[c],
                start=(c == 0),
                stop=(c == NCT - 1),
                skip_group_check=True,
            )
            if c == 3:
                yield 900
        qt = qt_pool.tile([128, 512], BF16, tag="qt", name=f"qt_{qc}_{p}")
        nc.vector.tensor_copy(qt, ps)
        qts_store.setdefault(qc, {})[p] = qt
        yield 950
        ps = ps_pool.tile([128, 512], F32, tag="ps", name=f"kps_{qc}_{p}")
        for c in range(NCT):
            nc.tensor.matmul(
                ps,
                wq_sb[:, c, FPC + p * 128:FPC + (p + 1) * 128],
                xs_tiles[c],
                start=(c == 0),
                stop=(c == NCT - 1),
                skip_group_check=True,
            )
            if c == 3:
                yield 900
        nc.vector.tensor_copy(kt[:, p, qc * 512:(qc + 1) * 512], ps)
        yield 950

    def v_item(qc, j):
        xs_tiles = xs_store[qc]
        kt_i = qc * 4 + j
        ps = ps_pool.tile([128, 512], F32, tag="ps", name=f"vps_{qc}_{j}")
        for c in range(NCT):
            nc.tensor.matmul(
                ps,
                xs_tiles[c][:, j * 128:(j + 1) * 128],
                wq_sb[:, c, 2 * FPC:3 * FPC],
                start=(c == 0),
                stop=(c == NCT - 1),
                skip_group_check=True,
            )
            if c == 3:
                yield 900
        nc.vector.tensor_scalar(
            out=v2[:, kt_i, :, 0:HD],
            in0=ps.rearrange("p (h d) -> p h d", h=HPC),
            scalar1=eacol[:, kt_i:kt_i + 1],
            scalar2=None,
            op0=mybir.AluOpType.mult,
        )
        yield 950

    def gen_chunk(qc):
        """One t-chunk's pre-attention work as resumable items, each yielding
        its approximate tensor-engine ns."""
        load_xs(qc)
        yield 0
        order = [qk_item(qc, 0), v_item(qc, 0), v_item(qc, 1),
                 qk_item(qc, 1), v_item(qc, 2), v_item(qc, 3),
                 qk_item(qc, 2), qk_item(qc, 3)]
        for it in order:
            yield from it

    def den_chain(qc, p, tmpA, tmpB):
        """Deferred denominator chain for pair (qc,p): spread-read, approx
        reciprocal (DVE), broadcast, then normalize multiplies on GPSIMD
        (whose in-order queue is the natural place to wait on its own DMA
        round trips).  Pulled one item per slot during the NEXT pair."""
        idx = qc * 4 + p
        dnp = rec_pool.tile([128, 8], F32, tag="dnp", name=f"dnp_{qc}_{p}")
        nc.gpsimd.dma_start(
            out=dnp, in_=dsc1[idx, :].rearrange("(j p) -> p j", p=128)
        )
        for _ in range(4):
            yield 0
        rcp = rec_pool.tile([128, 8], F32, tag="rcp", name=f"rcp_{qc}_{p}")
        nc.vector.reciprocal_approx_fast(out=rcp, in_=dnp)
        yield 0
        nc.gpsimd.dma_start(
            out=dsc2[idx, :].rearrange("(j p) -> p j", p=128), in_=rcp
        )
        yield 0
        sc = scale_pool.tile([64, 1024], F32, tag="sc", name=f"sc_{qc}_{p}")
        nc.gpsimd.dma_start(
            out=sc[0:64, :], in_=dsc2[idx:idx + 1, :].to_broadcast([64, 1024])
        )
        for _ in range(4):
            yield 0
        stack = stack_pool.tile([128, 512], BF16, tag="stack",
                                name=f"stk_{qc}_{p}")
        nc.gpsimd.tensor_mul(stack[0:64, :], tmpA[0:64, :], sc[0:64, 0:512])
        yield 0
        stkB = tmp_pool.tile([64, 512], BF16, tag="stkB", bufs=4,
                             name=f"skB_{qc}_{p}")
        nc.gpsimd.tensor_mul(stkB[0:64, :], tmpB[0:64, :], sc[0:64, 512:1024])
        nc.sync.dma_start(out=stack[64:128, :], in_=stkB[0:64, :])
        stacks_store.setdefault(qc, {})[p] = stack
        yield 0

    def gen_proj(qc):
        stacks = stacks_store[qc]
        pool = y_pool if qc == NQC - 1 else ps_pool
        tag = "y" if qc == NQC - 1 else "ps"
        for tq in range(4):
            row0 = qc * 512 + tq * 128
            for ch in range(2):
                ps = pool.tile([128, 512], F32, tag=tag,
                               name=f"pps_{qc}_{tq}_{ch}")
                for p in range(HPC // 2):
                    nc.tensor.matmul(
                        ps,
                        stacks[p][:, tq * 128:(tq + 1) * 128],
                        wp_sb[:, p, ch * 512:(ch + 1) * 512],
                        start=(p == 0),
                        stop=(p == HPC // 2 - 1),
                        skip_group_check=True,
                    )
                pout = pout_pool.tile([128, 512], F32, tag="pout",
                                      name=f"po_{qc}_{tq}_{ch}")
                nc.vector.tensor_copy(pout, ps)
                nc.sync.dma_start(
                    out=out_ap[row0:row0 + 128, ch * 512:(ch + 1) * 512],
                    in_=pout,
                )
                yield 1100

    # chunk 0: xs, pair-0 Q/K and all V upfront; remaining Q/K interleave
    # with chunk-0 attention as fillers (per-pair guard below)
    load_xs(0)
    for it in (qk_item(0, 0), v_item(0, 0), v_item(0, 1),
               v_item(0, 2), v_item(0, 3)):
        for _ in it:
            pass

    def gen_chunk0_rest():
        for it in (qk_item(0, 1), qk_item(0, 2), qk_item(0, 3)):
            yield from it

    deferred = []      # den chains: paced at most one item per pull call
    mains = [gen_chunk0_rest()]   # gen / proj generators: budget-filled
    proj_pending = []  # qc values whose proj hasn't been queued yet

    def pull_ns(budget):
        # 1. at most one deferred (den-chain) item per call
        if deferred:
            try:
                budget -= next(deferred[0]) or 0
            except StopIteration:
                deferred.pop(0)
        # 2. unlock proj(qc') once its stacks are all written
        if proj_pending and len(stacks_store.get(proj_pending[0], {})) == 4:
            mains.append(gen_proj(proj_pending.pop(0)))
        # 3. budget-fill from main generators, in order
        while budget > 0 and mains:
            try:
                est = next(mains[0])
                budget -= est if est is not None else 600
            except StopIteration:
                mains.pop(0)
                if (proj_pending
                        and len(stacks_store.get(proj_pending[0], {})) == 4):
                    mains.append(gen_proj(proj_pending.pop(0)))

    for qc in range(NQC):
        if qc + 1 < NQC:
            mains.append(gen_chunk(qc + 1))
        if qc > 0:
            proj_pending.append(qc - 1)

        nki = 4 * qc + 4
        for p in range(HPC // 2):
            while qts_store.get(qc, {}).get(p) is None:
                pull_ns(2000)
            qt = qts_store[qc][p]
            yA = y_pool.tile([128, 512], F32, tag="y", name=f"yA_{qc}_{p}")
            yB = y_pool.tile([128, 512], F32, tag="y", name=f"yB_{qc}_{p}")

            def n0_of(k):
                r = k - 4 * qc
                return 128 * r if r > 0 else 0

            ses = {}

            def emit_pv(k):
                n0p = n0_of(k)
                se3p = ses.pop(k)
                nc.tensor.matmul(
                    yA[0:HD + 1, n0p:512], v2[:, k, 2 * p, :],
                    se3p[:, 0, n0p:512],
                    start=(k == 0), stop=(k == nki - 1), skip_group_check=True,
                )
                nc.tensor.matmul(
                    yB[0:HD + 1, n0p:512], v2[:, k, 2 * p + 1, :],
                    se3p[:, 1, n0p:512],
                    start=(k == 0), stop=(k == nki - 1), skip_group_check=True,
                )
                return 2 * ((512 - n0p) / 2.4 + 20)

            for ki in range(nki):
                r = ki - 4 * qc
                n0 = n0_of(ki)
                tensor_ns = 0.0
                if ki >= 2:
                    tensor_ns += emit_pv(ki - 2)
                st = st_pool.tile([128, 1024], F32, tag="st",
                                  name=f"st_{qc}_{p}_{ki}")
                st3 = st.rearrange("p (h q) -> p h q", h=2)
                kslice = slice(ki * 128, (ki + 1) * 128)
                diag = r >= 0
                if diag:
                    # causal mask: seed the diagonal block with -1e30 above
                    # the diagonal (identity matmul); QK accumulates on top
                    nc.tensor.matmul(
                        st3[:, 0, n0:n0 + 128], id128, trineg,
                        start=True, stop=False, skip_group_check=True,
                    )
                    nc.tensor.matmul(
                        st3[:, 1, n0:n0 + 128], id128, trineg,
                        start=True, stop=False, skip_group_check=True,
                    )
                nc.tensor.matmul(
                    st3[:, 0, n0:512], kt[0:64, p, kslice], qt[0:64, n0:512],
                    start=not diag, stop=True, skip_group_check=True,
                )
                nc.tensor.matmul(
                    st3[:, 1, n0:512], kt[64:128, p, kslice], qt[64:128, n0:512],
                    start=not diag, stop=True, skip_group_check=True,
                )
                se = se_pool.tile([128, 1024], F32R, tag="se",
                                  name=f"se_{qc}_{p}_{ki}")
                se3 = se.rearrange("p (h q) -> p h q", h=2)
                nc.scalar.activation(
                    se3[:, :, n0:512], st3[:, :, n0:512], Exp, scale=SCALE
                )
                ses[ki] = se3

                tensor_ns += (512 - n0) / 2.4 + 25 + (110 if diag else 0)
                exp_ns = (2 * (512 - n0) + 352) / 1.2
                pull_ns(exp_ns - tensor_ns)

            emit_pv(nki - 2)
            pull_ns(500)
            emit_pv(nki - 1)

            # evacuate y^T + denominator rows (DVE; scalar stays pure exp)
            tmpA = tmp_pool.tile([128, 512], F32, tag="tmp", name=f"tmpA_{qc}_{p}")
            nc.vector.tensor_copy(tmpA[0:HD + 1, :], yA[0:HD + 1, :])
            tmpB = tmp_pool.tile([128, 512], F32, tag="tmp", name=f"tmpB_{qc}_{p}")
            nc.vector.tensor_copy(tmpB[0:HD + 1, :], yB[0:HD + 1, :])
            idx = qc * 4 + p
            nc.gpsimd.dma_start(out=dsc1[idx, 0:512], in_=tmpA[HD:HD + 1, :])
            nc.gpsimd.dma_start(out=dsc1[idx, 512:1024], in_=tmpB[HD:HD + 1, :])
            deferred.append(den_chain(qc, p, tmpA, tmpB))
            pull_ns(1200)

        # invariant: gen(qc+1) fully emitted before qc+1's attention
        # (paced so deferred den chains keep advancing through the dump)
        while mains:
            pull_ns(2000)

    # tail: drain remaining den chains back-to-back, then final projections
    while deferred:
        pull_ns(0)
    while proj_pending:
        qcq = proj_pending.pop(0)
        for _ in gen_proj(qcq):
            pass
    for _ in gen_proj(NQC - 1):
        pass


def make_nc():
    nc = bacc.Bacc("TRN2", target_bir_lowering=False, debug=False,
                   num_devices=NCORES)
    xT = nc.dram_tensor("xT", [C, T], BF16, kind="ExternalInput")
    wqkv = nc.dram_tensor("wqkv", [C, 3 * FPC], BF16, kind="ExternalInput")
    wproj = nc.dram_tensor("wproj", [FPC, C], BF16, kind="ExternalInput")
    ea = nc.dram_tensor("ea", [T], F32, kind="ExternalInput")
    out = nc.dram_tensor("out", [T, C], F32, kind="ExternalOutput")
    dsc1 = nc.dram_tensor("dsc1", [16, 1024], F32, kind="Internal")
    dsc2 = nc.dram_tensor("dsc2", [16, 1024], F32, kind="Internal")
    kq = np.arange(128)
    trineg_np = np.where(kq[:, None] <= kq[None, :], 0.0, NEG).astype(np.float32)
    trineg_dram = nc.inline_tensor(trineg_np, name="trineg_const")
    id_np = np.eye(128, dtype=np.float32)
    id_dram = nc.inline_tensor(id_np, name="id_const")
    with ExitStack() as ctx:
        tc = ctx.enter_context(tile.TileContext(nc))
        tc.ctx = ctx
        build(tc, out[:, :], xT[:, :], wqkv[:, :], wproj[:, :], ea[:],
              trineg_dram, id_dram, dsc1[:, :], dsc2[:, :])
    nc.compile()
    return nc


def shard_inputs(x, prev_probs, W_attn, W_proj):
    import ml_dtypes

    bf16 = ml_dtypes.bfloat16
    in_maps = []
    for core in range(NCORES):
        b, g = divmod(core, 2)
        xT = np.ascontiguousarray(x[b].T)
        wq = W_attn[:, g * FPC:(g + 1) * FPC]
        wk = W_attn[:, C + g * FPC:C + (g + 1) * FPC]
        wv = W_attn[:, 2 * C + g * FPC:2 * C + (g + 1) * FPC]
        wqkv = np.ascontiguousarray(np.concatenate([wq, wk, wv], axis=1))
        wproj = np.ascontiguousarray(W_proj[g * FPC:(g + 1) * FPC, :])
        ea = np.power(prev_probs[b] + np.float32(1e-10), np.float32(-EPS_BIAS))
        in_maps.append(
            {
                "xT": xT.astype(bf16),
                "wqkv": wqkv.astype(bf16),
                "wproj": wproj.astype(bf16),
                "ea": ea.astype(np.float32),
            }
        )
    return in_maps


_CACHED_NC = None


def kernel(x, prev_probs, W_attn, W_proj, trace=False, tmpdir=None):
    global _CACHED_NC
    from concourse.bass_utils import run_bass_kernel_spmd

    x = np.asarray(x, dtype=np.float32)
    prev_probs = np.asarray(prev_probs, dtype=np.float32)
    W_attn = np.asarray(W_attn, dtype=np.float32)
    W_proj = np.asarray(W_proj, dtype=np.float32)

    if _CACHED_NC is None:
        _CACHED_NC = make_nc()
    nc = _CACHED_NC

    in_maps = shard_inputs(x, prev_probs, W_attn, W_proj)
    res = run_bass_kernel_spmd(
        nc, in_maps, core_ids=list(range(NCORES)), trace=trace, tmpdir=tmpdir
    )
    parts = [r["out"] for r in res.results]
    out = np.empty((B, T, C), dtype=np.float32)
    for b in range(B):
        out[b] = parts[2 * b] + parts[2 * b + 1]
    kernel.last_results = res
    return out
